# revision 24
# baseline (speedup 1.0000x reference)
"""Contrastive loss on 8 Trainium2 NeuronCores (Bass/Tile).

loss * n = sum_ij [ same_ij * (s<1)(1-s) + (1-same_ij) * (s>0.3) * s ],
s = <x_i, x_j>.

Exact decomposition (rows pre-sorted by label so same-label pairs live in
a band |i-j| < pad):
    loss * n = sum_ij b(s) + sum_ij same_ij * (relu(1-s) - b(s)),
    b(s) = (s > 0.3) * s.

Strategy (vs. the full-S baseline):
  * S is symmetric -> only the upper triangle of the 16x16 grid of
    512-wide blocks is computed: 136 blocks instead of 256.  Core c is
    assigned triangle rows c and 15-c: (16-c) + (c+1) = 17 blocks for
    every core, including exactly two diagonal blocks.  Off-diagonal
    blocks enter the total with weight 2, diagonal blocks with weight 1.
  * One SPMD program: the host gathers, per core, per task slot
    t in 0..16, the lhsT block [128, 2, 512] and rhs block [128, 2, 512]
    into two resident fp8 tensors, so the program is task-index uniform.
    Slots 0/16 hold the two diagonal blocks, slots 1/15 the two
    band-corner blocks (r, r+1); the host also precomputes the
    label-equality masks for those four slots as bf16 tensors.
  * Matmul in fp8e4m3 with MatmulPerfMode.DoubleRow (K=256 in one pass;
    host-checked loss error ~8e-4, well inside the 2e-2 gate).
  * The PSUM->accumulator drain (the real bottleneck) is split across
    all three elementwise engines: per [128, 1024] S unit either
      C: ScalarE copy to SBUF bf16 + DVE fused (S>m)*S row-accumulated
         (4x DVE mode on bf16 SBUF operands), or
      V: DVE scalar_tensor_tensor directly on the PSUM tile, or
      P: Pool (gpsimd) scalar_tensor_tensor directly on the PSUM tile.
    Band-correction slots are pinned to mode C (they need S in SBUF).
  * Host: fp64 sum of per-core accumulator columns with weights 1/2,
    divide by n.
"""

import numpy as np
import ml_dtypes

import concourse.bass as bass
import concourse.mybir as mybir
from concourse import bacc
import concourse.tile as tile
from concourse.bass_utils import run_bass_kernel_spmd

N_TOTAL = 8192
D = 256
N_CORES = 8
GB = 512                      # grid block width
G = N_TOTAL // GB             # 16 col/row blocks
NS = 17                       # task slots per core
ST = 4                        # 128-row stripes per block
MARGIN = 0.3
F32 = mybir.dt.float32
BF16 = mybir.dt.bfloat16
FP8 = mybir.dt.float8e4

# Per-stripe drain units: groups of task slots whose PSUM tiles sit in
# adjacent banks and are drained by ONE wide op pair. The (0,16) pair
# holds the two diagonal blocks (weight 1, band corrections); quadA
# stripe 3 holds the two band-corner slots. 2048-wide ops halve the
# per-op sync overhead vs 1024.
UNITS = [
    ("pair", (0, 16), 1024, 1.0),
    ("quadA", (1, 15, 2, 3), 2048, 2.0),
    ("quadB", (4, 5, 6, 7), 2048, 2.0),
    ("quadC", (8, 9, 10, 11), 2048, 2.0),
    ("tri", (12, 13, 14), 1536, 2.0),
]
NU = len(UNITS)
# drain engine per (unit, stripe): 'A' = Act relu(s-m)-copy w/ accum +
# DVE count; 'V' = DVE max(s,m) tensor_scalar w/ accum + DVE count.
# Band unit-stripes (pair all stripes, quadA stripe 3) are forced to a
# plain Act copy + masked window zeroing + DVE max/count pair.
UNIT_ENGINE = {
    "pair": "AAAA",
    "quadA": "AAAA",
    "quadB": "AAAA",
    "quadC": "AAVV",
    "tri": "AAAA",
}


def unit_kind(ui, st):
    name = UNITS[ui][0]
    if name == "pair" or (name == "quadA" and st == 3):
        return "band"
    return "relu" if UNIT_ENGINE[name][st] == "A" else "max"


def _windows(pad):
    """Band windows for a diagonal block, per stripe: (lo, hi) col range
    within the 512-wide block plus running offset into the mask tensor."""
    wins = []
    off = 0
    for st in range(ST):
        lo = max(0, 128 * st - pad)
        hi = min(GB, 128 * st + 128 + pad)
        wins.append((st, lo, hi - lo, off))
        off += hi - lo
    return wins, off            # off = 512 + 6*pad


def build_program(pad, repeats=1, ablate=frozenset()):
    """ablate (timing experiments only, breaks math): 'nocnt' drop count
    ops, 'nocopy' drop drains, 'nomm' drop matmuls."""
    assert 0 < pad <= 96
    nc = bacc.Bacc()
    LW = NS * GB                # 8704 cols in lhs/rhs tensors
    lhs_d = nc.dram_tensor("lhs8", [128, 2, LW], FP8, kind="ExternalInput")
    rhs_d = nc.dram_tensor("rhs8", [128, 2, LW], FP8, kind="ExternalInput")
    wins, wlen = _windows(pad)
    MKW = 2 * wlen + 2 * pad    # masks: slot0 | slot1 | slot15 | slot16
    mk_d = nc.dram_tensor("mk", [128, MKW], BF16, kind="ExternalInput")
    m_off = {0: 0, 1: wlen, 15: wlen + pad, 16: wlen + 2 * pad}

    n_base = 2 * NU * ST        # (sum, count) per unit-stripe
    corr_cols = {}              # (slot, st) -> col
    cc = n_base
    for slot in (0, 16):
        for st in range(ST):
            corr_cols[(slot, st)] = cc
            cc += 1
    for slot in (1, 15):
        corr_cols[(slot, 3)] = cc
        cc += 1
    CD = cc
    out_d = nc.dram_tensor("out", [128, CD], F32, kind="ExternalOutput")

    AL = mybir.AluOpType
    ACT = mybir.ActivationFunctionType
    DR = mybir.MatmulPerfMode.DoubleRow

    with tile.TileContext(nc) as tc:
        with (
            tc.tile_pool(name="resident", bufs=1) as rpool,
            tc.tile_pool(name="psum", bufs=2, space="PSUM") as psum,
            tc.tile_pool(name="scopy", bufs=3) as spool,
            tc.tile_pool(name="junk", bufs=3) as jpool,
            tc.tile_pool(name="band", bufs=2) as wpool,
        ):
            lhs8 = rpool.tile([128, 2, LW], FP8, name="lhs8")
            rhs8 = rpool.tile([128, 2, LW], FP8, name="rhs8")
            for chunk in range(4):
                sl = slice(chunk * (LW // 4), (chunk + 1) * (LW // 4))
                nc.sync.dma_start(out=lhs8[:, :, sl], in_=lhs_d[:, :, sl])
                nc.sync.dma_start(out=rhs8[:, :, sl], in_=rhs_d[:, :, sl])
            mk = rpool.tile([128, MKW], BF16, name="mk")
            nc.sync.dma_start(out=mk[:], in_=mk_d[:])
            imk = rpool.tile([128, MKW], BF16, name="imk")
            nc.vector.tensor_scalar(
                out=imk[:], in0=mk[:], scalar1=0.5, scalar2=None,
                op0=AL.is_lt,
            )
            bias_nm = rpool.tile([128, 1], F32, name="bias_nm")
            nc.vector.memset(bias_nm[:], -MARGIN)

            accD = rpool.tile([128, CD], F32, name="accD")
            nc.vector.memset(accD[:], 0.0)
            acc_ap = lambda col: accD[:, col:col + 1]

            def mm(dst, slot, st):
                nc.tensor.matmul(
                    dst,
                    lhs8[:, :, slot * GB + st * 128: slot * GB + (st + 1) * 128],
                    rhs8[:, :, slot * GB: (slot + 1) * GB],
                    start=True, stop=True, perf_mode=DR,
                )

            def count_op(src_tile, width, col, thresh):
                if "nocnt" in ablate:
                    return
                jc = jpool.tile([128, width], BF16, name="cnt")
                nc.vector.tensor_scalar(
                    out=jc[:], in0=src_tile[:, 0:width], scalar1=thresh,
                    scalar2=None, op0=AL.is_gt, op1=AL.add,
                    accum_out=acc_ap(col),
                )

            def body():
                for st in range(ST):
                    for ui, (name, slots, width, _w) in enumerate(UNITS):
                        T = psum.tile([128, 2048], F32, name="S")
                        if "nomm" not in ablate:
                            for h, slot in enumerate(slots):
                                mm(T[:, h * GB:(h + 1) * GB], slot, st)
                        if "nocopy" in ablate:
                            continue
                        col = 2 * (ui * ST + st)
                        kind = unit_kind(ui, st)
                        Tv = T[:, 0:width]
                        if kind == "relu":
                            Sb = spool.tile([128, width], BF16, name="scp")
                            nc.scalar.activation(
                                out=Sb[:], in_=Tv, func=ACT.Relu,
                                bias=bias_nm[:], scale=1.0,
                                accum_out=acc_ap(col),
                            )
                            count_op(Sb, width, col + 1, 0.0)
                        elif kind == "max":
                            Sb = spool.tile([128, width], BF16, name="scp")
                            nc.vector.tensor_scalar(
                                out=Sb[:], in0=Tv, scalar1=MARGIN,
                                scalar2=None, op0=AL.max, op1=AL.add,
                                accum_out=acc_ap(col),
                            )
                            count_op(Sb, width, col + 1, 0.3015)
                        else:
                            # band: plain copy, window corrections + zero,
                            # then max/count over the modified tile
                            Sb = spool.tile([128, width], BF16, name="scp")
                            nc.scalar.activation(
                                out=Sb[:], in_=Tv, func=ACT.Copy,
                                bias=0.0, scale=1.0,
                            )
                            if name == "pair":
                                winlist = [(0, 0, wins[st][1], wins[st][2],
                                            wins[st][3]),
                                           (16, 1, wins[st][1], wins[st][2],
                                            wins[st][3])]
                            else:
                                winlist = [(1, 0, 0, pad, 0),
                                           (15, 1, 0, pad, 0)]
                            for slot, half, lo, w, moff in winlist:
                                sl = slice(half * GB + lo, half * GB + lo + w)
                                msl = slice(m_off[slot] + moff,
                                            m_off[slot] + moff + w)
                                at = wpool.tile([128, w], BF16, name="at")
                                nc.scalar.activation(
                                    out=at[:], in_=Sb[:, sl], func=ACT.Relu,
                                    bias=1.0, scale=-1.0,
                                )
                                jw = wpool.tile([128, w], BF16, name="jw")
                                nc.vector.scalar_tensor_tensor(
                                    out=jw[:], in0=at[:], scalar=0.0,
                                    in1=mk[:, msl], op0=AL.add, op1=AL.mult,
                                    accum_out=acc_ap(corr_cols[(slot, st)]),
                                )
                                nc.vector.tensor_tensor(
                                    out=Sb[:, sl], in0=Sb[:, sl],
                                    in1=imk[:, msl], op=AL.mult,
                                )
                            jm = jpool.tile([128, width], BF16, name="jm")
                            nc.vector.tensor_scalar(
                                out=jm[:], in0=Sb[:], scalar1=MARGIN,
                                scalar2=None, op0=AL.max, op1=AL.add,
                                accum_out=acc_ap(col),
                            )
                            count_op(jm, width, col + 1, 0.3015)

            import contextlib
            loop_cm = tc.For_i(0, repeats, 1) if repeats > 1 else \
                contextlib.nullcontext()
            with loop_cm:
                body()

            nc.sync.dma_start(out=out_d[:], in_=accD[:])

    meta = dict(CD=CD, n_base=n_base, corr_cols=dict(corr_cols), pad=pad)
    return nc, meta


def host_reduce(out_arr, meta):
    """out_arr: [128, CD] f32 from one core -> fp64 partial of loss*n."""
    a = out_arr.astype(np.float64)
    tot = 0.0
    for ui, (name, slots, width, w) in enumerate(UNITS):
        for st in range(ST):
            c0 = 2 * (ui * ST + st)
            s0, s1 = a[:, c0].sum(), a[:, c0 + 1].sum()
            if unit_kind(ui, st) == "relu":
                part = s0 + MARGIN * s1
            else:
                part = s0 + MARGIN * (s1 - 128.0 * width)
            tot += w * part
    for (slot, st), col in meta["corr_cols"].items():
        w = 1.0 if slot in (0, 16) else 2.0
        tot += w * a[:, col].sum()
    return tot


def task_slots(c):
    """Slot -> (row block, col block) for core c. Slots 0/16 diagonal,
    1/15 band-corner; the rest hold the remaining triangle blocks."""
    rA, rB = c, (G - 1) - c
    blocks = [(rA, j) for j in range(rA, G)] + \
             [(rB, j) for j in range(rB, G)]
    slots = {0: (rA, rA), 1: (rA, rA + 1), 16: (rB, rB)}
    if c >= 1:
        slots[15] = (rB, rB + 1)
    fixed = set(slots.values())
    rest = [blk for blk in blocks if blk not in fixed]
    free = [s for s in range(NS) if s not in slots]
    for s, blk in zip(free, rest, strict=True):
        slots[s] = blk
    return slots


def prepare_inputs(inputs, targets):
    X = np.asarray(inputs, dtype=np.float32)
    t = np.asarray(targets).astype(np.int64).reshape(-1)
    n, d = X.shape
    assert (n, d) == (N_TOTAL, D), f"kernel hardcoded for {N_TOTAL}x{D}"
    perm = np.argsort(t, kind="stable")
    ts_ = t[perm]
    bounds = np.flatnonzero(
        np.concatenate(([True], ts_[1:] != ts_[:-1], [True])))
    maxrun = int(np.diff(bounds).max())
    pad = int(-(-max(32, maxrun) // 32) * 32)
    XT = np.ascontiguousarray(X[perm].T).astype(ml_dtypes.float8_e4m3)
    # [128, 2, N]: partition lane p holds dims p (k0) and 128+p (k1)
    XK = XT.reshape(2, 128, N_TOTAL).transpose(1, 0, 2)
    tf = ts_.astype(np.float64)
    wins, wlen = _windows(pad)
    MKW = 2 * wlen + 2 * pad

    in_maps = []
    for c in range(N_CORES):
        slots = task_slots(c)
        lhs = np.zeros((128, 2, NS * GB), dtype=XK.dtype)
        rhs = np.zeros((128, 2, NS * GB), dtype=XK.dtype)
        for s in range(NS):
            r, j = slots[s]
            lhs[:, :, s * GB:(s + 1) * GB] = XK[:, :, r * GB:(r + 1) * GB]
            rhs[:, :, s * GB:(s + 1) * GB] = XK[:, :, j * GB:(j + 1) * GB]
        mkv = np.zeros((128, MKW), dtype=np.float64)
        off = 0
        for slot in (0, 1, 15, 16):
            if slot in (0, 16):
                r, j = slots[slot]
                for st, lo, w, moff in wins:
                    rows = tf[r * GB + st * 128: r * GB + (st + 1) * 128]
                    cols = tf[j * GB + lo: j * GB + lo + w]
                    mkv[:, off + moff: off + moff + w] = (
                        rows[:, None] == cols[None, :])
                off += wlen
            else:
                if slot in slots:
                    r, j = slots[slot]
                    rows = tf[r * GB + 3 * 128: r * GB + 4 * 128]
                    cols = tf[j * GB: j * GB + pad]
                    mkv[:, off: off + pad] = (rows[:, None] == cols[None, :])
                off += pad
        in_maps.append({
            "lhs8": lhs,
            "rhs8": rhs,
            "mk": mkv.astype(ml_dtypes.bfloat16),
        })
    return in_maps, pad


def run(inputs, targets, trace=False):
    in_maps, pad = prepare_inputs(inputs, targets)
    nc, meta = build_program(pad)
    nc.finalize()
    res = run_bass_kernel_spmd(
        nc, in_maps, core_ids=list(range(N_CORES)), trace=trace
    )
    total = 0.0
    for r in res.results:
        total += host_reduce(r["out"], meta)
    return np.asarray(total / N_TOTAL, dtype=np.float32), res


def kernel(inputs, targets):
    val, _ = run(inputs, targets, trace=False)
    return val
